# revision 45
# baseline (speedup 1.0000x reference)
"""Trainium2 Bass kernel for nn_AttentionBlock (sparse attention with gaussian bias).

Reference computation (per batch b):
    qp = q @ Wq + bq; kp = k @ Wk + bk; vp = v @ Wv + bv          (d_model=512 -> dk=dv=64)
    attn = qp @ kp^T / 8 + g_bias / (2 tau^2); attn[mask] = -inf
    p = softmax(attn, axis=-1)
    out = (p @ vp) @ Wfc + bfc

Sharding: 8 cores = (batch b in 0..3) x (query-half h in 0..1); K/V replicated
(no collectives). Each core computes a [1024 q, 2048 k] attention slab.

Transposed-scores design: all big operands arrive HOST-TRANSPOSED so no PE
transposes are needed anywhere:
  - qT/kT/vT [512, rows] f16; gmT [2048 k, 1024 q] fp8-e5m2 with mask folded in
    on host (masked = -57344 -> exp ~ 0).
  - qpT[64, 1024] = Wq^T qT (scaled by 2 tau^2/8, +bq'), kpT[64, 2048] = Wk^T kT
    + bk, vp[128k, 65] = (v Wv_aug + bv_aug) with ones column 64 (rowsum trick).
  - per k-tile: sT[128k, 1024q] = kpT_tile^T @ qpT  (+ ident @ gmT accumulate),
    eT[:, kt, :] = exp(sT * escale - 3)  (f16; the -3 shift cancels in softmax).
  - PV: oT_aug[65, 512q] = sum_kt matmul(lhsT=vp[kt], rhs=eT[kt]); row 64 =
    rowsum.
  - FC per 128-q chunk: out = (aoT^T @ Wfc_aug) * recip(rowsum) + bfc, with
    rowsum extracted via a [65,1] selector matmul.

DMA strategy: every input rides ONE sync-engine HWDGE ring, sequenced in
consumption order (qT; then per kT chunk its gmT pairs; vT last) — per-ring
FIFO draining makes this a free priority scheduler and avoids cross-queue
bandwidth sharing. Constants are packed into 3 DMAs on the scalar ring.
Dummy matmuls on a zero tile warm the PE HAM clock gate during the qT load.
"""
import numpy as np
import ml_dtypes

B, S, D, DKV = 4, 2048, 512, 64
DV1 = DKV + 1          # vp augmented with ones column
SQL = S // 2           # query rows per core
NKT = S // 128         # 16 k tiles
N_CORES = 8
MASK_VAL = -57344.0    # max-magnitude finite fp8-e5m2; * 1/(2 tau^2) -> -31.9
PC1 = 512 + DV1 + 4    # f32 const pack cols: bfc | bv_aug | bq bk qs es
PC2 = 128 + 128 + DV1  # f16 W pack cols: Wq,Wq | Wk,Wk | Wv_aug (q/k duplicated for row-split scores)
N_WARMUP = 18


def _build():
    import concourse.bass as bass
    import concourse.mybir as mybir
    import concourse.tile as tile
    from concourse import bacc

    f32, f16, f8 = mybir.dt.float32, mybir.dt.float16, mybir.dt.float8e5
    AF = mybir.ActivationFunctionType
    OP = mybir.AluOpType
    MM_DR = mybir.MatmulPerfMode.DoubleRow

    nc = bacc.Bacc(num_devices=N_CORES)
    qT_ext = nc.declare_dram_parameter("qT", [D, SQL], f16, isOutput=False)
    kT_ext = nc.declare_dram_parameter("kT", [D, S], f16, isOutput=False)
    vT_ext = nc.declare_dram_parameter("vT", [D, S], f16, isOutput=False)
    gmT_ext = nc.declare_dram_parameter("gmT", [S, SQL], f8, isOutput=False)
    p1_ext = nc.declare_dram_parameter("p1", [128, PC1], f32, isOutput=False)
    p2_ext = nc.declare_dram_parameter("p2", [D, PC2], f16, isOutput=False)
    p3_ext = nc.declare_dram_parameter("p3", [DV1, D + 1], f16, isOutput=False)
    out_ext = nc.declare_dram_parameter("out", [SQL, D], f32, isOutput=True)

    with tile.TileContext(nc) as tc:
        from contextlib import ExitStack
        with ExitStack() as ctx:
            wpool = ctx.enter_context(tc.tile_pool(name="weights", bufs=1))
            ppool = ctx.enter_context(tc.tile_pool(name="persist", bufs=1))

            # ---- constants: zero scratch (for PE warmup) + packed const DMAs
            zs_t = wpool.tile([128, 512], f16, tag="zs")
            nc.gpsimd.memset(zs_t[:], 0.0)
            eb_t = wpool.tile([128, 1], f32, tag="eb")
            nc.gpsimd.memset(eb_t[:], -3.0)

            p1_t = wpool.tile([128, PC1], f32, tag="p1")
            p2_t = wpool.tile([128, 4, PC2], f16, tag="p2")
            p3_t = wpool.tile([DV1, D + 1], f16, tag="p3")
            nc.scalar.dma_start(p1_t[:], p1_ext[:])
            nc.scalar.dma_start(p2_t[:], p2_ext.rearrange("(j p) n -> p j n", p=128))
            nc.scalar.dma_start(p3_t[:], p3_ext[:])
            bfc_t = p1_t[:, 0:512]
            bv_t = p1_t[:, 512:512 + DV1]
            bq_t = p1_t[:, PC1 - 4:PC1 - 3]
            bk_t = p1_t[:, PC1 - 3:PC1 - 2]
            qs_t = p1_t[:, PC1 - 2:PC1 - 1]
            es_t = p1_t[:, PC1 - 1:PC1]
            wfc_t = p3_t[:, 0:D]
            sel_t = p3_t[:, D:D + 1]

            ident = wpool.tile([128, 128], f32, tag="ident")
            ident8 = wpool.tile([128, 128], f8, tag="ident8")
            from concourse.masks import make_identity
            make_identity(nc, ident[:])
            nc.gpsimd.tensor_copy(ident8[:], ident[:])
            # DoubleRow identity pairs: (ident, 0) and (0, ident) so one fp8
            # DoubleRow matmul adds a 512-col half of gmT into PSUM.
            id_dr = [wpool.tile([128, 2, 128], f8, tag=f"idr{o}", name=f"idr{o}")
                     for o in range(2)]
            for o in range(2):
                nc.gpsimd.memset(id_dr[o][:], 0.0)
                nc.gpsimd.tensor_copy(id_dr[o][:, o, :], ident8[:])

            # ---- persistent projected tensors ----
            qpT = ppool.tile([128, SQL], f16, tag="qpT")
            kpT = ppool.tile([128, S], f16, tag="kpT")
            vp_sb = ppool.tile([128, NKT, DV1], f16, tag="vp")
            eT = ppool.tile([128, NKT, SQL], f16, tag="eT")
            aoT = ppool.tile([DV1, SQL], f16, tag="aoT")
            rc_t = ppool.tile([128, SQL // 128], f32, tag="recip")

            with tc.tile_pool(name="pv_ps", bufs=1, space="PSUM") as pv_ps:
                pv_tiles = [pv_ps.tile([DV1, 512], f32, tag=f"pv{g}", name=f"pv{g}")
                            for g in range(2)]

                with tc.tile_pool(name="q_in", bufs=1) as q_in, \
                     tc.tile_pool(name="k_in", bufs=4) as k_in, \
                     tc.tile_pool(name="v_in", bufs=2) as v_in, \
                     tc.tile_pool(name="gm_in", bufs=8) as gm_in, \
                     tc.tile_pool(name="ps_proj", bufs=2, space="PSUM") as ps_proj, \
                     tc.tile_pool(name="ps_sc", bufs=2, space="PSUM") as ps_sc:

                    # All inputs on the sync HWDGE ring in consumption order.
                    q_sb = q_in.tile([128, 4, SQL], f16, tag="q")
                    for qh in range(2):
                        nc.sync.dma_start(
                            q_sb[:, :, 512 * qh:512 * (qh + 1)],
                            qT_ext[:, 512 * qh:512 * (qh + 1)]
                            .rearrange("(j p) n -> p j n", p=128))
                    k_sbs, gm_in_tiles = [], []
                    for c in range(4):
                        k_sb = k_in.tile([128, 4, 512], f16, tag="kc", name=f"ksb{c}")
                        nc.sync.dma_start(
                            k_sb[:], kT_ext[:, 512 * c:512 * (c + 1)]
                            .rearrange("(j p) n -> p j n", p=128))
                        k_sbs.append(k_sb)
                        for gp in (2 * c, 2 * c + 1):
                            gm_sb = gm_in.tile([128, 2, 2, SQL // 2], f8, tag="gm",
                                               name=f"gm{gp}")
                            nc.sync.dma_start(
                                gm_sb[:], gmT_ext[256 * gp:256 * (gp + 1), :]
                                .rearrange("(t p) (o n) -> p t o n", p=128, o=2))
                            gm_in_tiles.append(gm_sb)
                    vt_sbs = []
                    for c2 in range(2):
                        vt_sb = v_in.tile([128, 4, SQL], f16, tag="vt", name=f"vsb{c2}")
                        nc.sync.dma_start(
                            vt_sb[:], vT_ext[:, SQL * c2:SQL * (c2 + 1)]
                            .rearrange("(j p) n -> p j n", p=128))
                        vt_sbs.append(vt_sb)

                    # PE warmup: keep HAM busy while qT streams in
                    for w in range(N_WARMUP):
                        pw = ps_proj.tile([128, 512], f32, tag="pp", name=f"warm{w}")
                        nc.tensor.matmul(pw[:], zs_t[:, :128], zs_t[:],
                                         start=True, stop=True)

                    # Q projection: qpT[64, 1024] = sum_j Wq_j^T @ qT_j, scaled
                    for c in range(SQL // 512):
                        pp = ps_proj.tile([128, 512], f32, tag="pp")
                        for j in range(4):
                            nc.tensor.matmul(pp[:], p2_t[:, j, 0:128],
                                             q_sb[:, j, 512 * c:512 * (c + 1)],
                                             start=(j == 0), stop=(j == 3))
                        nc.vector.tensor_scalar(
                            out=qpT[:, 512 * c:512 * (c + 1)], in0=pp[:],
                            scalar1=bq_t, scalar2=qs_t, op0=OP.add, op1=OP.mult)

                    # Streamed K projection + scores: per 512-col kT chunk,
                    # project kpT then run the 4 dependent score k-tiles.
                    for c in range(4):
                        pp = ps_proj.tile([128, 512], f32, tag="pp")
                        for j in range(4):
                            nc.tensor.matmul(pp[:], p2_t[:, j, 128:256],
                                             k_sbs[c][:, j, :],
                                             start=(j == 0), stop=(j == 3))
                        nc.vector.tensor_scalar(
                            out=kpT[:, 512 * c:512 * (c + 1)],
                            in0=pp[:], scalar1=bk_t, scalar2=None, op0=OP.add)

                        for kt in range(4 * c, 4 * c + 4):
                            ps = ps_sc.tile([128, SQL], f32, tag="sc")
                            ksl = slice(128 * kt, 128 * (kt + 1))
                            gm_sb = gm_in_tiles[kt // 2]
                            for g in range(2):
                                qsl = slice(512 * g, 512 * (g + 1))
                                hp = slice(64 * g, 64 * g + 64)
                                nc.tensor.matmul(ps[:, qsl], kpT[hp, ksl],
                                                 qpT[hp, qsl],
                                                 start=True, stop=False)
                            for g in range(2):
                                qsl = slice(512 * g, 512 * (g + 1))
                                nc.tensor.matmul(ps[:, qsl], id_dr[g][:],
                                                 gm_sb[:, kt % 2, :, :],
                                                 start=False, stop=True,
                                                 perf_mode=MM_DR)
                            nc.scalar.activation(eT[:, kt, :], ps[:], AF.Exp,
                                                 bias=eb_t[:], scale=es_t)

                    # V projection: vp[128, 65] per k-tile (+ones col)
                    for kt in range(NKT):
                        pv = ps_proj.tile([128, 512], f32, tag="pp")
                        vloc = kt % 8
                        for j in range(4):
                            nc.tensor.matmul(
                                pv[:, :DV1],
                                vt_sbs[kt // 8][:, j, 128 * vloc:128 * (vloc + 1)],
                                p2_t[:, j, 256:256 + DV1],
                                start=(j == 0), stop=(j == 3))
                        nc.vector.tensor_tensor(out=vp_sb[:, kt, :], in0=pv[:, :DV1],
                                                in1=bv_t, op=OP.add)

                    # PV: oT_aug[65, 512] per q-group, accumulated over k-tiles
                    for g in range(2):
                        qsl = slice(512 * g, 512 * (g + 1))
                        for kt in range(NKT):
                            nc.tensor.matmul(pv_tiles[g][:], vp_sb[:, kt, :],
                                             eT[:, kt, qsl],
                                             start=(kt == 0), stop=(kt == NKT - 1))
                        nc.vector.tensor_copy(aoT[:, qsl], pv_tiles[g][:])

            # ---- FC + normalize + store ----
            with tc.tile_pool(name="out_sb", bufs=3) as out_pool, \
                 tc.tile_pool(name="ps_fc", bufs=2, space="PSUM") as ps_fc, \
                 tc.tile_pool(name="ps_rs", bufs=2, space="PSUM") as ps_rs:
                for qc in range(SQL // 128):
                    rs = ps_rs.tile([128, 1], f32, tag="rs")
                    nc.tensor.matmul(rs[:], aoT[:, 128 * qc:128 * (qc + 1)],
                                     sel_t, start=True, stop=True)
                    nc.vector.reciprocal(rc_t[:, qc:qc + 1], rs[:])
                for qc in range(SQL // 128):
                    qsl = slice(128 * qc, 128 * (qc + 1))
                    fc = ps_fc.tile([128, D], f32, tag="fc")
                    nc.tensor.matmul(fc[:], aoT[:, qsl], wfc_t,
                                     start=True, stop=True)
                    o_sb = out_pool.tile([128, D], f32, tag="osb")
                    nc.vector.scalar_tensor_tensor(
                        out=o_sb[:], in0=fc[:], scalar=rc_t[:, qc:qc + 1],
                        in1=bfc_t, op0=OP.mult, op1=OP.add)
                    eng = nc.scalar if qc % 2 == 0 else nc.sync
                    eng.dma_start(out_ext[qsl, :], o_sb[:])

    nc.finalize()
    return nc


_cache = {}


def kernel(**inputs):
    from concourse.bass_utils import run_bass_kernel_spmd

    q = np.asarray(inputs["q"], np.float32)
    k = np.asarray(inputs["k"], np.float32)
    v = np.asarray(inputs["v"], np.float32)
    gb = np.asarray(inputs["g_bias"], np.float32)
    mask = np.asarray(inputs["mask"]).astype(bool)
    tau = float(np.asarray(inputs["tau"]))

    if "nc" not in _cache:
        _cache["nc"] = _build()
    nc = _cache["nc"]

    in_maps = build_in_maps(inputs, q, k, v, gb, mask, tau)
    res = run_bass_kernel_spmd(nc, in_maps, list(range(N_CORES)))
    out = np.empty((B, S, D), np.float32)
    for c in range(N_CORES):
        b, h = divmod(c, 2)
        out[b, h * SQL:(h + 1) * SQL] = res.results[c]["out"]
    return out


def build_in_maps(inputs, q, k, v, gb, mask, tau):
    f16 = np.float16
    f8 = ml_dtypes.float8_e5m2
    Wv = np.asarray(inputs["Wv"], np.float32)
    Wfc = np.asarray(inputs["Wfc"], np.float32)
    p1 = np.zeros((128, PC1), np.float32)
    p1[:, 0:512] = np.asarray(inputs["bfc"], np.float32)
    p1[:, 512:512 + DKV] = np.asarray(inputs["bv"], np.float32)
    p1[:, 512 + DKV] = 1.0
    bq2 = np.tile(np.asarray(inputs["bq"], np.float32), 2)
    bk2 = np.tile(np.asarray(inputs["bk"], np.float32), 2)
    p1[:, PC1 - 4] = bq2
    p1[:, PC1 - 3] = bk2
    p1[:, PC1 - 2] = (2.0 * tau * tau) / 8.0
    p1[:, PC1 - 1] = 1.0 / (2.0 * tau * tau)
    p2 = np.zeros((D, PC2), np.float32)
    Wq = np.asarray(inputs["Wq"], np.float32)
    Wk = np.asarray(inputs["Wk"], np.float32)
    p2[:, 0:DKV] = Wq
    p2[:, DKV:128] = Wq
    p2[:, 128:128 + DKV] = Wk
    p2[:, 128 + DKV:256] = Wk
    p2[:, 256:256 + DKV] = Wv
    p3 = np.zeros((DV1, D + 1), np.float32)
    p3[:DKV, 0:D] = Wfc
    p3[DKV, D] = 1.0
    shared = {"p1": p1, "p2": p2.astype(f16), "p3": p3.astype(f16)}
    mask = np.asarray(mask).astype(bool)
    in_maps = []
    for c in range(N_CORES):
        b, h = divmod(c, 2)
        sl = slice(h * SQL, (h + 1) * SQL)
        gmT = np.where(mask[b, sl].T, np.float32(MASK_VAL), gb[b, sl].T)
        in_maps.append({
            "qT": np.ascontiguousarray(q[b, sl].T.astype(f16)),
            "kT": np.ascontiguousarray(k[b].T.astype(f16)),
            "vT": np.ascontiguousarray(v[b].T.astype(f16)),
            "gmT": np.ascontiguousarray(gmT.astype(f8)),
            **shared,
        })
    return in_maps
